# revision 30
# baseline (speedup 1.0000x reference)
"""Trainium2 Bass kernel for the BSplineLayer (KAN-style) problem.

y = einsum('oic,bic->bo', coeffs, Bspline(clip(x))) + silu(x) @ W.T + x

Algebraic reduction: the spline grid is uniform and identical for every
in_dim, and x is clipped to (-1, 1). Restricted to that interval each of the
13 cubic B-spline basis functions is a cubic spline whose only interior knots
are {-0.8, -0.4, 0, 0.4, 0.8} — a 9-dim function space spanned by
{1, v, v^2, v^3, relu(+/-(v-s))^3}. The 13->9 change of basis is folded into
`coeffs` on the host, so the device computes 8 cheap elementwise feature
planes (+ a silu plane) and one K = 512*9 matmul, with the constant term via
a K=1 ones-row matmul and the residual added during the PSUM drain.

The matmul runs in float32r (tf32, full PE rate). To recover fp32-level
accuracy, weights are hi/lo split on the host (free) and feature planes are
hi/lo split on device, giving W.P ~= Wh.Ph + Wl.Ph + Wh.Pl (the lo.lo term
is ~2^-22 relative). The two +/-0.8 truncated-cube blocks contribute < 3e-6
relative error unrounded, so their correction passes are skipped.

Layout: transposed throughout (in/out features on partitions, batch on the
free dim). Each of the 8 cores takes a 1024-row batch shard; weights are
replicated. y^T shards are gathered and transposed on the host.
"""

import os
from contextlib import ExitStack

import numpy as np

import concourse.bass as bass
import concourse.bacc as bacc
import concourse.tile as tile
from concourse import mybir
from concourse.bass_utils import run_bass_kernel_spmd

# ---- problem constants (must match the grader's reference) ----
BATCH, IN_DIM, OUT_DIM = 8192, 512, 512
GRID_SIZE, SPLINE_ORDER = 5, 3
N_BASES = 2 * GRID_SIZE + SPLINE_ORDER  # 13
H = 2.0 / GRID_SIZE  # 0.4
CLIP_LO = float(-1.0 + 1e-4)
CLIP_HI = float(1.0 - 1e-4)
INNER_KNOTS = (-0.8, -0.4, 0.0, 0.4, 0.8)
SIDES = (-1.0, -1.0, 1.0, 1.0, 1.0)  # truncation side per knot (small support)

N_CORES = 8
BPC = BATCH // N_CORES  # 1024 batch rows per core
NT = 512  # matmul moving free-dim tile
NCH = BPC // NT  # 2
NBLK = IN_DIM // 128  # 4 i-blocks
NM = 9  # feature planes: v, v^2, v^3, 5 trunc cubes, silu
# planes whose hi/lo correction passes run (all but the +/-0.8 truncs)
CORR = (0, 1, 2, 4, 5, 6, 8)
NCORR = len(CORR)

F32 = mybir.dt.float32
F32R = mybir.dt.float32r
AF = mybir.ActivationFunctionType
ALU = mybir.AluOpType

LAST_EXEC_NS = None


# ------------------------- host-side math -------------------------

def _tf32_round(a):
    """Round-to-nearest-even to tf32 (10-bit mantissa), matching fp32r."""
    u = np.ascontiguousarray(a, np.float32).view(np.uint32).copy()
    rb = ((u >> 13) & 1).astype(np.uint32)
    u += np.uint32(0x0FFF) + rb
    u &= np.uint32(0xFFFFE000)
    return u.view(np.float32)


def _bspline_f64(v):
    """Exact de Boor recursion in f64 on the uniform grid (the reference's
    1e-8 denominator eps is a no-op in f32 and negligible in f64)."""
    g = np.arange(-GRID_SIZE - SPLINE_ORDER, GRID_SIZE + SPLINE_ORDER + 1,
                  dtype=np.float64) * H
    b = ((v[:, None] >= g[None, :-1]) & (v[:, None] < g[None, 1:])).astype(np.float64)
    for k in range(1, SPLINE_ORDER + 1):
        d1 = g[k:-1] - g[:-(k + 1)]
        left = (v[:, None] - g[None, :-(k + 1)]) / d1[None, :]
        d2 = g[k + 1:] - g[1:-k]
        right = (g[None, k + 1:] - v[:, None]) / d2[None, :]
        b = left * b[:, :-1] + right * b[:, 1:]
    return b  # [n, 13]


def _features_f64(v):
    """[n, 9]: 1, v, v^2, v^3, then the 5 one-sided truncated cubes."""
    cols = [np.ones_like(v), v, v ** 2, v ** 3]
    for s, sg in zip(INNER_KNOTS, SIDES):
        cols.append(np.maximum(sg * (v - s), 0.0) ** 3)
    return np.stack(cols, axis=1)


def _basis_change():
    """A [13, 9] with B_c(v) = sum_m A[c, m] f_m(v) on the clipped interval."""
    v = np.linspace(CLIP_LO, CLIP_HI, 8001)
    M = _features_f64(v)
    B = _bspline_f64(v)
    A, _, _, _ = np.linalg.lstsq(M, B, rcond=None)
    return A.T  # [13, 9]


_A = _basis_change()


def _fold_weights(coeffs, base_weight):
    """Returns (wh [NBLK,128,NM*OUT], wl [NBLK,128,NCORR*OUT], bias hi/lo)."""
    C2 = np.einsum('oic,cm->oim', coeffs.astype(np.float64), _A)  # [O, I, 9]
    bias = C2[:, :, 0].sum(axis=1)  # [O]
    W_all = np.concatenate(
        [C2[:, :, 1:], base_weight.astype(np.float64)[:, :, None]], axis=2
    )  # [O, I, 9]
    W = np.transpose(W_all, (1, 2, 0))  # [I, 9, O]
    Wh = _tf32_round(W.astype(np.float32))
    Wl = _tf32_round((W - Wh.astype(np.float64)).astype(np.float32))
    wh = np.ascontiguousarray(Wh.reshape(NBLK, 128, NM * OUT_DIM))
    wl = np.ascontiguousarray(
        Wl[:, list(CORR), :].reshape(NBLK, 128, NCORR * OUT_DIM))
    bh = _tf32_round(bias.astype(np.float32))
    bl = _tf32_round((bias - bh.astype(np.float64)).astype(np.float32))
    brow = np.stack([bh, bl], axis=0).reshape(2, OUT_DIM)
    return wh, wl, brow


# ------------------------- device kernel -------------------------

def _emit_kernel(ctx: ExitStack, tc: tile.TileContext, yt, xt, wh, wl, brow,
                 fast: bool):
    nc = tc.nc
    corr = () if fast else CORR

    whpool = ctx.enter_context(tc.tile_pool(name="wh", bufs=2))
    wlpool = ctx.enter_context(tc.tile_pool(name="wl", bufs=2))
    xpool = ctx.enter_context(tc.tile_pool(name="x", bufs=1))
    php = ctx.enter_context(tc.tile_pool(name="ph", bufs=2))
    plp = ctx.enter_context(tc.tile_pool(name="plo", bufs=1))
    tpool = ctx.enter_context(tc.tile_pool(name="tmp", bufs=2))
    cpool = ctx.enter_context(tc.tile_pool(name="const", bufs=1))
    pspool = ctx.enter_context(tc.tile_pool(name="ps", bufs=1, space="PSUM"))
    opool = ctx.enter_context(tc.tile_pool(name="out", bufs=2))

    # constants
    ones_f = cpool.tile([1, BPC], F32, tag="ones_f")
    nc.gpsimd.memset(ones_f[:], 1.0)
    ones = cpool.tile([1, BPC], F32R, tag="ones")
    nc.vector.tensor_copy(ones[:], ones_f[:])
    bts = []
    for hl in range(2):
        t = cpool.tile([1, OUT_DIM], F32R, tag=f"bt{hl}", name=f"bt{hl}")
        nc.sync.dma_start(t[:], brow[hl:hl + 1, :])
        bts.append(t)

    _consts = {}

    def const_col(val):
        """[128, 1] per-partition constant for ACT bias operands."""
        val = float(val)
        if val not in _consts:
            t = cpool.tile([128, 1], F32, tag=f"c{len(_consts)}",
                           name=f"c{len(_consts)}")
            nc.gpsimd.memset(t[:], val)
            _consts[val] = t
        return _consts[val][:]

    # x^T shard, resident (silu input + residual), chunked for DMA overlap
    xts = {}
    for ib in range(NBLK):
        for nch in range(NCH):
            t = xpool.tile([128, NT], F32, tag=f"xt{ib}_{nch}",
                           name=f"xt{ib}_{nch}")
            nc.sync.dma_start(t[:], xt[ib * 128:(ib + 1) * 128,
                                       nch * NT:(nch + 1) * NT])
            xts[(ib, nch)] = t

    pss = {}
    for ot in range(4):
        for nch in range(NCH):
            pss[(ot, nch)] = pspool.tile([128, NT], F32, tag=f"ps{ot}_{nch}",
                                         name=f"ps{ot}_{nch}")

    # bias rows go in first (K=1 matmuls against ones): they only need a tiny
    # DMA, so the PE starts (and HAM warms) while x/W are still loading
    for ot in range(4):
        for nch in range(NCH):
            for hl in range(2):
                nc.tensor.matmul(
                    pss[(ot, nch)][:], bts[hl][0:1, ot * 128:ot * 128 + 128],
                    ones[0:1, nch * NT:(nch + 1) * NT],
                    start=(hl == 0), stop=False)

    for ib in range(NBLK):
        # W streamed per i-block, chunked per-m so the first matmuls don't
        # wait for the whole block
        whts = []
        for m in range(NM):
            t = whpool.tile([128, OUT_DIM], F32R, tag=f"wh{m}",
                            name=f"wh{ib}_{m}")
            nc.sync.dma_start(t[:], wh[ib, :, m * OUT_DIM:(m + 1) * OUT_DIM])
            whts.append(t)
        wlts = []
        if corr:
            for k in range(NCORR):
                t = wlpool.tile([128, OUT_DIM], F32R, tag=f"wl{k}",
                                name=f"wl{ib}_{k}")
                nc.sync.dma_start(t[:],
                                  wl[ib, :, k * OUT_DIM:(k + 1) * OUT_DIM])
                wlts.append(t)

        for nch in range(NCH):
            xtb = xts[(ib, nch)]

            # ---- full-precision feature planes [128, NT] ----
            praw = {}

            def raw(m, name):
                praw[m] = tpool.tile([128, NT], F32, tag="raw", bufs=5,
                                     name=f"{name}{ib}_{nch}")
                return praw[m]

            sgm = tpool.tile([128, NT], F32, tag="sgm", name=f"sgm{ib}_{nch}")
            nc.scalar.activation(sgm[:], xtb[:], AF.Sigmoid,
                                 bias=const_col(0.0))
            nc.gpsimd.tensor_tensor(raw(8, "sil")[:], sgm[:], xtb[:],
                                    ALU.mult)

            v = raw(0, "v")
            nc.vector.tensor_scalar(v[:], xtb[:], CLIP_LO, CLIP_HI,
                                    ALU.max, ALU.min)
            v2 = raw(1, "v2")
            nc.vector.tensor_tensor(v2[:], v[:], v[:], ALU.mult)
            v3 = raw(2, "v3")
            nc.vector.tensor_tensor(v3[:], v2[:], v[:], ALU.mult)

            # rounded (tf32) planes the matmuls consume
            ph = {}
            for j, (s, sg) in enumerate(zip(INNER_KNOTS, SIDES)):
                m = 3 + j
                r = tpool.tile([128, NT], F32, tag="r", bufs=3,
                               name=f"r{j}_{ib}_{nch}")
                nc.scalar.activation(r[:], v[:], AF.Relu,
                                     bias=const_col(-sg * s), scale=float(sg))
                if j == 2:
                    q = v2
                elif j == 4:
                    q = tpool.tile([128, NT], F32, tag="q", bufs=3,
                                   name=f"q4_{ib}_{nch}")
                    nc.vector.tensor_tensor(q[:], r[:], r[:], ALU.mult)
                else:
                    q = tpool.tile([128, NT], F32, tag="q", bufs=3,
                                   name=f"q{j}_{ib}_{nch}")
                    nc.scalar.activation(q[:], v[:], AF.Square,
                                         bias=const_col(-s))
                eng = nc.gpsimd if j in (0, 1) else nc.vector
                if m in corr:
                    f = raw(m, f"f{j}")
                    eng.tensor_tensor(f[:], q[:], r[:], ALU.mult)
                else:
                    # uncorrected plane: write tf32 directly
                    t = php.tile([128, NT], F32R, tag=f"ph{m}",
                                 name=f"ph{m}_{ib}_{nch}")
                    eng.tensor_tensor(t[:], q[:], r[:], ALU.mult)
                    ph[m] = t

            # hi (tf32-rounding copy) and lo (residual) of raw planes
            pl = {}
            for k, m in enumerate(sorted(praw)):
                t = php.tile([128, NT], F32R, tag=f"ph{m}",
                             name=f"ph{m}_{ib}_{nch}")
                ceng = (nc.vector, nc.gpsimd, nc.scalar)[k % 3]
                if ceng is nc.scalar:
                    nc.scalar.activation(t[:], praw[m][:], AF.Identity,
                                         bias=const_col(0.0))
                else:
                    ceng.tensor_copy(t[:], praw[m][:])
                ph[m] = t
                if m in corr:
                    lo = plp.tile([128, NT], F32R, tag=f"pl{m}",
                                  name=f"pl{m}_{ib}_{nch}")
                    seng = nc.vector if k % 2 == 0 else nc.gpsimd
                    seng.tensor_tensor(lo[:], praw[m][:], t[:], ALU.subtract)
                    pl[m] = lo

            # ---- matmuls into the 4 o-tiles of this n-chunk ----
            # mains first (only need wh + ph), corrections after (wl, pl).
            # In the last i-block go o-tile-major so each PSUM bank finishes
            # early and its drain overlaps the remaining matmuls.
            osl = lambda ot: slice(ot * 128, ot * 128 + 128)
            last = (ib == NBLK - 1)
            if not last:
                for m in range(NM):
                    for ot in range(4):
                        nc.tensor.matmul(
                            pss[(ot, nch)][:], whts[m][:, osl(ot)], ph[m][:],
                            start=False, stop=False)
                for k, m in enumerate(corr):
                    for ot in range(4):
                        nc.tensor.matmul(pss[(ot, nch)][:],
                                         wlts[k][:, osl(ot)],
                                         ph[m][:], start=False, stop=False)
                        nc.tensor.matmul(pss[(ot, nch)][:],
                                         whts[m][:, osl(ot)],
                                         pl[m][:], start=False, stop=False)
            else:
                for ot in range(4):
                    ps = pss[(ot, nch)][:]
                    for m in range(NM):
                        nc.tensor.matmul(
                            ps, whts[m][:, osl(ot)], ph[m][:], start=False,
                            stop=(not corr and m == NM - 1))
                    for k, m in enumerate(corr):
                        nc.tensor.matmul(ps, wlts[k][:, osl(ot)], ph[m][:],
                                         start=False, stop=False)
                        nc.tensor.matmul(
                            ps, whts[m][:, osl(ot)], pl[m][:],
                            start=False, stop=(k == len(corr) - 1))
                    # drain: residual add + store
                    yo = opool.tile([128, NT], F32, tag="yo",
                                    name=f"yo{ot}_{nch}")
                    nc.vector.tensor_tensor(yo[:], ps, xts[(ot, nch)][:],
                                            ALU.add)
                    nc.sync.dma_start(
                        yt[ot * 128:(ot + 1) * 128,
                           nch * NT:(nch + 1) * NT], yo[:])


_NC_CACHE = {}


def _build(fast=False):
    if fast in _NC_CACHE:
        return _NC_CACHE[fast]
    nc = bacc.Bacc("TRN2", target_bir_lowering=False, debug=False,
                   num_devices=N_CORES)
    xt = nc.dram_tensor("xt", [IN_DIM, BPC], F32, kind="ExternalInput").ap()
    wh = nc.dram_tensor("wh", [NBLK, 128, NM * OUT_DIM], F32R,
                        kind="ExternalInput").ap()
    wl = nc.dram_tensor("wl", [NBLK, 128, NCORR * OUT_DIM], F32R,
                        kind="ExternalInput").ap()
    brow = nc.dram_tensor("brow", [2, OUT_DIM], F32R, kind="ExternalInput").ap()
    yt = nc.dram_tensor("yt", [OUT_DIM, BPC], F32, kind="ExternalOutput").ap()
    with tile.TileContext(nc) as tc, ExitStack() as ctx:
        _emit_kernel(ctx, tc, yt, xt, wh, wl, brow, fast)
    nc.compile()
    _NC_CACHE[fast] = nc
    return nc


def kernel(x, coeffs, base_weight):
    global LAST_EXEC_NS
    x = np.ascontiguousarray(x, dtype=np.float32)
    wh, wl, brow = _fold_weights(np.asarray(coeffs, np.float32),
                                 np.asarray(base_weight, np.float32))
    fast = bool(int(os.environ.get("KERNEL_FAST", "0")))
    nc = _build(fast)

    in_maps = []
    for c in range(N_CORES):
        shard = np.ascontiguousarray(x[c * BPC:(c + 1) * BPC, :].T)
        in_maps.append({"xt": shard, "wh": wh, "wl": wl, "brow": brow})

    trace = bool(int(os.environ.get("KERNEL_TRACE", "0")))
    res = run_bass_kernel_spmd(nc, in_maps, core_ids=list(range(N_CORES)),
                               trace=trace)
    LAST_EXEC_NS = res.exec_time_ns

    y = np.empty((BATCH, OUT_DIM), dtype=np.float32)
    for c in range(N_CORES):
        y[c * BPC:(c + 1) * BPC, :] = res.results[c]["yt"].T
    return y
